# revision 19
# baseline (speedup 1.0000x reference)
"""HOSVD aggregator kernel for 8 TRN2 NeuronCores.

y[n,o] = sum_{m0..m4} G[m0,m1,m2,m3,m4] * ris0[n,m0] * ris1[n,m4]
         * ris2[n,m3] * ris3[n,m2] * U_out[m1,o],
with ris_d = X[:,d,:] @ U_stack[d].

Strategy: data-parallel over nodes (6250/core). All compute in a
"transposed" layout (features on SBUF partitions, nodes on the free dim)
so every step is a TensorE matmul or a VectorE elementwise multiply and
no on-chip transposes are needed. Host pre-packs X as bf16 with channels
on partitions, and pre-expands the factor matrices:
  A01[n,a]=ris0[n,a//8], B01[n,a]=ris1[n,a%8]  (a=(m0,m4), 64)
  z01 = A01*B01;  A23t/B23t tiled to 128 rows, z23t = A23t*B23t
  PT[n,(m1 b)] = z01 @ G2,  G2[(m0 m4),(m1,m3,m2)] = G.transpose(0,4,1,3,2)
  Q = PT * tile(z23);  y = Q @ repeat(U_out, 64, axis=0)

The device loop is software-pipelined: TensorE block s issues
factors(s), PT(s-1), y(s-2) so the ScalarE/DVE stages between matmul
stages never stall the PE array. One Q-multiply runs on Pool (via a
ScalarE PSUM stage), the rest on DVE. Output is bf16, widened on host.
"""

import sys

sys.path.insert(0, "/opt/trn_rl_repo")

import os
import numpy as np
import ml_dtypes

import concourse.bass as bass
import concourse.tile as tile
from concourse import mybir
from concourse.bass_utils import run_bass_kernel_spmd

BF16 = ml_dtypes.bfloat16

N = 50000
NCORES = 8
NPC = N // NCORES            # 6250 nodes per core
T = 512                      # nodes per supertile
NSUP = NPC // T              # 12 full supertiles
TAIL = NPC - NSUP * T        # 106
NST = NSUP + 1               # total supertiles incl. tail

# ---------------------------------------------------------------------------
# walrus rejects >1 sync wait on a Drain; Tile's tail drain carries one wait
# per logical proc. Split it into a chain of single-wait drains.
import bass_rust as _br
from concourse.vector_clock import ScopedClock as _ScopedClock


def _split_drain_and_barrier(self, tick_clock, wait_clock):
    drain_inst = self.nc.sync.drain()
    wait_clock.add_sem_waits(
        drain_inst.ins, _ScopedClock({None: tick_clock.global_clock})
    )
    si = drain_inst.ins.sync_info
    waits = list(si.on_wait)
    if len(waits) > 1:
        drain_inst.ins.sync_info = _br.SyncInfo(on_wait=waits[:1], on_update=[])
        rest = waits[1:]
        while rest:
            d2 = self.nc.sync.drain()
            chunk, rest = rest[:1], rest[1:]
            d2.ins.sync_info = _br.SyncInfo(
                on_wait=chunk, on_update=list(si.on_update) if not rest else []
            )
    self.nc.all_engine_barrier()
    assert self.sems is not None
    popped = self.nc._tile_sem_poison_stack.pop()
    assert popped is self._sem_poison
    self.nc.clear_and_free_semaphores(list(self.sems.allocated().values()))
    self.nc.all_engine_barrier()


tile.TileContext._drain_and_barrier = _split_drain_and_barrier

# Same walrus limit applies to every instruction type: peel extra sem waits
# onto single-wait NOPs emitted just before the instruction, same engine.
_SPLIT_SEQ = [0]
_orig_add_instruction = tile.TileContext._add_instruction


def _split_add_instruction(self, inst):
    si = inst.sync_info
    waits = list(si.on_wait) if si is not None else []
    if len(waits) > 1:
        for w in waits[:-1]:
            _SPLIT_SEQ[0] += 1
            nop = mybir.InstNoOp(name=f"waitsplit_{_SPLIT_SEQ[0]}", ins=[],
                                 outs=[], engine=inst.engine)
            nop.sync_info = _br.SyncInfo(on_wait=[w], on_update=[])
            _orig_add_instruction(self, nop)
        inst.sync_info = _br.SyncInfo(on_wait=[waits[-1]],
                                      on_update=list(si.on_update))
    return _orig_add_instruction(self, inst)


tile.TileContext._add_instruction = _split_add_instruction

# ---------------------------------------------------------------------------
# weight-pack free-dim offsets (all bf16, one [128, 1792] SBUF tile)
_W0 = 0      # 2 chunks [128,64]  : A01 factor (repeat U0 cols 8x)
_W1 = 128    # 2 chunks [128,64]  : B01 factor (tile U1 cols 8x)
_W2 = 256    # 2 chunks [128,128] : A23t factor
_W3 = 512    # 2 chunks [128,128] : B23t factor
_G2 = 768    # [64,512] rows 0-63 : core matrix, 4 lhsT chunks of [64,128]
_UE = 1280   # 4 chunks [128,128] : U_out expanded over b
_WCOLS = 1792
_WFAC = 768  # first DMA piece covers all factor weight chunks


def _build_nc():
    nc = bass.Bass("TRN2", target_bir_lowering=False, debug=False,
                   num_devices=NCORES)
    bf = mybir.dt.bfloat16
    f32 = mybir.dt.float32

    xm = nc.dram_tensor("xm", [NSUP, 128, 8 * T], bf, kind="ExternalInput").ap()
    xt = nc.dram_tensor("xt", [128, 8 * TAIL], bf, kind="ExternalInput").ap()
    wp = nc.dram_tensor("wp", [128, _WCOLS], bf, kind="ExternalInput").ap()
    ym = nc.dram_tensor("ym", [NSUP, 128, T], bf, kind="ExternalOutput").ap()
    yt = nc.dram_tensor("yt", [128, TAIL], bf, kind="ExternalOutput").ap()

    def colcount(s):
        return T if s < NSUP else TAIL

    ORD = list(range(NSUP)) + [NSUP]   # full supertiles, tail last

    with tile.TileContext(nc) as tc:
        from contextlib import ExitStack
        with ExitStack() as ctx:
            wpool = ctx.enter_context(tc.tile_pool(name="w", bufs=1))
            xpool = ctx.enter_context(tc.tile_pool(name="x", bufs=4))
            spool = ctx.enter_context(tc.tile_pool(name="s", bufs=2))
            qpool = ctx.enter_context(tc.tile_pool(name="q", bufs=8))
            ypool = ctx.enter_context(tc.tile_pool(name="y", bufs=2))
            pfac = ctx.enter_context(tc.tile_pool(name="pf", bufs=1, space="PSUM"))
            ppt = ctx.enter_context(tc.tile_pool(name="ppt", bufs=4, space="PSUM"))
            pyp = ctx.enter_context(tc.tile_pool(name="py", bufs=1, space="PSUM"))

            # startup: factor weights first, then x supertiles in order
            ws = wpool.tile([128, _WCOLS], bf)
            xtiles = {}
            nc.sync.dma_start(ws[:, 0:_WFAC], wp[:, 0:_WFAC])
            for i in range(min(3, NST)):
                s = ORD[i]
                tc_ = colcount(s)
                xs = xpool.tile([128, 8 * tc_], bf, tag="xs")
                nc.sync.dma_start(xs[:], xm[s] if s < NSUP else xt[:])
                xtiles[s] = xs
                if i == 0:
                    nc.sync.dma_start(ws[:, _WFAC:_WCOLS], wp[:, _WFAC:_WCOLS])

            state = {}   # per-supertile live tiles

            for blk in range(NST + 2):
                s = ORD[blk] if blk < NST else -1          # factors stage
                sp = ORD[blk - 1] if 1 <= blk <= NST else -1   # PT stage
                sy = ORD[blk - 2] if 2 <= blk <= NST + 1 else -1  # y stage

                # prefetch xs three blocks ahead
                if blk + 3 < NST:
                    f = ORD[blk + 3]
                    tc_ = colcount(f)
                    xs = xpool.tile([128, 8 * tc_], bf, tag="xs")
                    nc.sync.dma_start(xs[:], xm[f] if f < NSUP else xt[:])
                    xtiles[f] = xs

                # ---- factors(s): 8 matmuls + scalar staging + z muls ----
                if s >= 0:
                    tc_ = colcount(s)
                    xs = xtiles.pop(s)

                    def xc(d, c):
                        return xs[:, (2 * d + c) * tc_:(2 * d + c + 1) * tc_]

                    ps01 = pfac.tile([128, T], f32, tag="ps01")
                    if 1 <= blk <= 2:
                        # keep the PE p-state warm across the pipeline-fill
                        # DMA wait; result is overwritten by the start=True
                        # matmul below
                        for _w in range(2):
                            nc.tensor.matmul(ps01[0:64, :], ws[:, _W0:_W0 + 64],
                                             ws[:, 0:T], start=True, stop=True)
                    nc.tensor.matmul(ps01[0:64, :tc_], ws[:, _W0:_W0 + 64],
                                     xc(0, 0), start=True, stop=False)
                    nc.tensor.matmul(ps01[0:64, :tc_], ws[:, _W0 + 64:_W0 + 128],
                                     xc(0, 1), start=False, stop=True)
                    nc.tensor.matmul(ps01[64:128, :tc_], ws[:, _W1:_W1 + 64],
                                     xc(1, 0), start=True, stop=False,
                                     tile_position=(0, 64))
                    nc.tensor.matmul(ps01[64:128, :tc_], ws[:, _W1 + 64:_W1 + 128],
                                     xc(1, 1), start=False, stop=True,
                                     tile_position=(0, 64))
                    psA = pfac.tile([128, T], f32, tag="psA")
                    nc.tensor.matmul(psA[:, :tc_], ws[:, _W2:_W2 + 128], xc(2, 0),
                                     start=True, stop=False)
                    nc.tensor.matmul(psA[:, :tc_], ws[:, _W2 + 128:_W2 + 256],
                                     xc(2, 1), start=False, stop=True)
                    psB = pfac.tile([128, T], f32, tag="psB")
                    nc.tensor.matmul(psB[:, :tc_], ws[:, _W3:_W3 + 128], xc(3, 0),
                                     start=True, stop=False)
                    nc.tensor.matmul(psB[:, :tc_], ws[:, _W3 + 128:_W3 + 256],
                                     xc(3, 1), start=False, stop=True)

                    # stage one PSUM operand through SBUF on ScalarE (PSUM has
                    # a single DVE read port; avoid dual-PSUM tensor_tensor)
                    a01s = spool.tile([64, T], f32, tag="a01s")
                    nc.scalar.copy(a01s[:, :tc_], ps01[0:64, :tc_])
                    z01 = spool.tile([64, T], bf, tag="z01")
                    nc.vector.tensor_mul(z01[:, :tc_], ps01[64:128, :tc_],
                                         a01s[:, :tc_])
                    a23s = spool.tile([128, T], f32, tag="a23s")
                    nc.scalar.copy(a23s[:, :tc_], psA[:, :tc_])
                    z23 = spool.tile([128, T], bf, tag="z23")
                    nc.vector.tensor_mul(z23[:, :tc_], psB[:, :tc_],
                                         a23s[:, :tc_])
                    state[s] = {"z01": z01, "z23": z23, "tc": tc_}

                # ---- PT(sp) + Q(sp): 4 matmuls, muls on DVE/Pool ----
                if sp >= 0:
                    st = state[sp]
                    tc_ = st["tc"]
                    z01, z23 = st["z01"], st["z23"]
                    qts = []
                    for qq in range(4):
                        ptq = ppt.tile([128, T], f32, tag="pt", name="ptq")
                        nc.tensor.matmul(ptq[:, :tc_],
                                         ws[0:64, _G2 + 128 * qq:_G2 + 128 * (qq + 1)],
                                         z01[:, :tc_], start=True, stop=True)
                        qt = qpool.tile([128, T], bf, tag="qt", name="qt")
                        if qq < 3:
                            nc.vector.tensor_mul(qt[:, :tc_], ptq[:, :tc_],
                                                 z23[:, :tc_])
                        else:
                            qs3 = spool.tile([128, T], f32, tag="qs3", name="qs3")
                            nc.scalar.copy(qs3[:, :tc_], ptq[:, :tc_])
                            nc.gpsimd.tensor_mul(qt[:, :tc_], qs3[:, :tc_],
                                                 z23[:, :tc_])
                        qts.append(qt)
                    st["qts"] = qts

                # ---- y(sy): 4 accumulating matmuls + copy + DMA out ----
                if sy >= 0:
                    st = state.pop(sy)
                    tc_ = st["tc"]
                    psy = pyp.tile([128, T], f32, tag="psy", name="psy")
                    for qq in range(4):
                        nc.tensor.matmul(psy[:, :tc_],
                                         ws[:, _UE + 128 * qq:_UE + 128 * (qq + 1)],
                                         st["qts"][qq][:, :tc_],
                                         start=(qq == 0), stop=(qq == 3))
                    ys = ypool.tile([128, T], bf, tag="ys", name="ys")
                    nc.scalar.copy(ys[:, :tc_], psy[:, :tc_])
                    nc.sync.dma_start(ym[sy] if sy < NSUP else yt[:],
                                      ys[:, :tc_])

    return nc


def _host_pack_weights(G, U_stack, U_output):
    U = np.asarray(U_stack, np.float32)
    Uo = np.asarray(U_output, np.float32)
    Gf = np.asarray(G, np.float32)
    wpk = np.zeros((128, _WCOLS), np.float32)
    W0 = np.repeat(U[0], 8, axis=1)            # [256,64]
    W1 = np.tile(U[1], (1, 8))                 # [256,64]
    W2 = np.tile(np.repeat(U[2], 8, axis=1), (1, 2))   # [256,128]
    W3 = np.tile(U[3], (1, 16))                # [256,128]
    wpk[:, _W0:_W0 + 64] = W0[:128]
    wpk[:, _W0 + 64:_W0 + 128] = W0[128:]
    wpk[:, _W1:_W1 + 64] = W1[:128]
    wpk[:, _W1 + 64:_W1 + 128] = W1[128:]
    wpk[:, _W2:_W2 + 128] = W2[:128]
    wpk[:, _W2 + 128:_W2 + 256] = W2[128:]
    wpk[:, _W3:_W3 + 128] = W3[:128]
    wpk[:, _W3 + 128:_W3 + 256] = W3[128:]
    G2 = np.ascontiguousarray(Gf.transpose(0, 4, 1, 3, 2)).reshape(64, 512)
    wpk[0:64, _G2:_G2 + 512] = G2
    Uexp = np.repeat(Uo, 64, axis=0)           # [512,128]
    for q in range(4):
        wpk[:, _UE + 128 * q:_UE + 128 * (q + 1)] = Uexp[128 * q:128 * (q + 1)]
    return wpk.astype(BF16)


def _install_ntff_hook():
    import types
    if "antenv.axon_hooks" in sys.modules:
        return
    mod = types.ModuleType("antenv.axon_hooks")
    holder = {"hook": None}
    mod.set_axon_ntff_profile_hook = lambda h: holder.__setitem__("hook", h)
    mod.get_axon_ntff_profile_hook = lambda: holder["hook"]
    sys.modules["antenv.axon_hooks"] = mod
    import antenv
    antenv.axon_hooks = mod
    from trn_agent_boot.trn_boot import _ntff_profile_via_ctypes
    mod.set_axon_ntff_profile_hook(_ntff_profile_via_ctypes("/opt/axon/libaxon_pjrt.so"))


_NC_CACHE = None


def kernel(neighbour_states, G, U_stack, U_output):
    global _NC_CACHE
    X = np.asarray(neighbour_states, np.float32)
    wpb = _host_pack_weights(G, U_stack, U_output)

    in_maps = []
    for c in range(NCORES):
        sh = X[c * NPC:(c + 1) * NPC]                      # [6250, 4, 256]
        main = (sh[:NSUP * T]
                .reshape(NSUP, T, 4, 2, 128)
                .transpose(0, 4, 2, 3, 1)                  # [s, p, d, ch, t]
                .reshape(NSUP, 128, 8 * T))
        tail = (sh[NSUP * T:]
                .reshape(TAIL, 4, 2, 128)
                .transpose(3, 1, 2, 0)
                .reshape(128, 8 * TAIL))
        in_maps.append({
            "xm": np.ascontiguousarray(main).astype(BF16),
            "xt": np.ascontiguousarray(tail).astype(BF16),
            "wp": wpb,
        })

    if _NC_CACHE is None:
        _NC_CACHE = _build_nc()
    nc = _NC_CACHE

    trace = bool(os.environ.get("HOSVD_TRACE"))
    if trace:
        _install_ntff_hook()
    res = run_bass_kernel_spmd(nc, in_maps, core_ids=list(range(NCORES)),
                               trace=trace)
    if trace and res.exec_time_ns is not None:
        print(f"HW exec time: {res.exec_time_ns} ns")

    out = np.empty((N, 128), np.float32)
    for c in range(NCORES):
        ymc = np.asarray(res.results[c]["ym"]).astype(np.float32)
        ytc = np.asarray(res.results[c]["yt"]).astype(np.float32)
        base = c * NPC
        out[base:base + NSUP * T] = ymc.transpose(0, 2, 1).reshape(NSUP * T, 128)
        out[base + NSUP * T:base + NPC] = ytc.T
    return out
